# revision 9
# baseline (speedup 1.0000x reference)
"""Trainium2 kernel for gnn_message_passing (nn_MessagePassing_41480794145043).

out = relu((segment_mean of node_features[edge_src] over edge_dst) @ (W+B)^T)

Strategy (8 NeuronCores, SPMD single program):
  - Destination nodes sharded across cores (25000 dst rows each).
  - Host: sorts each core's edges by (src_bucket, dst_window128), pads each
    (bucket, window) group to a multiple of 128 with identical group sizes on
    every core so one static program serves all cores.
  - Device: per 8192-edge chunk, dma_gather (GatherAnt, bf16 table padded to
    256B rows) pulls source features; a single broadcast is_equal builds all
    the [128e x 128d] one-hot tiles; one matmul per 128-edge subtile
    accumulates agg^T windows in PSUM (all 196 windows resident).
  - Epilogue per window: PSUM->SBUF, PE transpose, small matmul with (W+B)^T,
    scale by 1/max(count,1), relu, one big DMA out.
"""
import sys
import json
import numpy as np

sys.path.insert(0, '/opt/trn_rl_repo')

import ml_dtypes  # noqa: E402
import concourse.bass as bass  # noqa: E402
import concourse.mybir as mybir  # noqa: E402
import concourse.tile as tile  # noqa: E402
from concourse import bacc  # noqa: E402

BF16 = ml_dtypes.bfloat16

N = 200000
E = 12800000
D = 10
NCORES = 8
NPC = N // NCORES            # dst nodes per core
WIN = 128                    # dst window (matmul M)
NW = (NPC + WIN - 1) // WIN  # windows per core (196 for 25000)
BK = 32768                   # src bucket size (int16 index range)
NB = (N + BK - 1) // BK      # 7 buckets
CH = 8192                    # edges per gather chunk
TBL_ROWS = NB * BK           # padded feature table rows
ES = 128                     # gathered elem size in bf16 (= 256B)
WPB = 51                     # windows per PSUM bank (51*10=510 <= 512 fp32)
DEBUG_NO_LINEAR = False
DEBUG_NO_GATHER = False


# ----------------------------------------------------------------- BIR fix --
def _fix_bir_json(bir_json: bytes) -> bytes:
    """This walrus build accepts max one sem-wait per instruction; split
    multi-wait instructions into preceding single-wait EventSemaphore nops."""
    d = json.loads(bir_json)
    for fn in d.get("functions", []):
        for bb in fn.get("blocks", []):
            out = []
            for ins in bb.get("instructions", []):
                sync = ins.get("sync_info")
                waits = (sync or {}).get("on_wait") or []
                if len(waits) > 1:
                    for i, w in enumerate(waits[:-1]):
                        out.append({
                            "debug": ins.get("debug", 0),
                            "engine": ins["engine"],
                            "ins": [], "outs": [],
                            "name": f"{ins['name']}-wsplit{i}",
                            "opcode": "EventSemaphore",
                            "sync_info": {"on_update": [], "on_wait": [w]},
                        })
                    sync["on_wait"] = [waits[-1]]
                out.append(ins)
            bb["instructions"] = out
    return json.dumps(d).encode()


def _install_bir_fix():
    import concourse.bass_utils as bu
    import concourse.bass2jax as b2j
    orig = bu.compile_bir_kernel
    if getattr(orig, "_bir_fix_installed", False):
        return
    def wrapped(bir_json, tmpdir, neff_name="file.neff"):
        return orig(_fix_bir_json(bir_json), tmpdir, neff_name=neff_name)
    wrapped._bir_fix_installed = True
    bu.compile_bir_kernel = wrapped
    b2j.compile_bir_kernel = wrapped


# ------------------------------------------------------------------ runner --
class _Runner:
    def __init__(self, nc, n_cores):
        import jax
        from jax.sharding import Mesh, PartitionSpec
        from jax.experimental.shard_map import shard_map
        from concourse.bass2jax import (_bass_exec_p, install_neuronx_cc_hook,
                                        partition_id_tensor)
        install_neuronx_cc_hook()
        self.jax = jax
        self.n_cores = n_cores
        in_names, out_names, out_avals, zero_outs = [], [], [], []
        pname = nc.partition_id_tensor.name if nc.partition_id_tensor else None
        for alloc in nc.m.functions[0].allocations:
            if not isinstance(alloc, mybir.MemoryLocationSet):
                continue
            name = alloc.memorylocations[0].name
            if alloc.kind == "ExternalInput":
                if name != pname:
                    in_names.append(name)
            elif alloc.kind == "ExternalOutput":
                shape = list(alloc.tensor_shape)
                np_dt = mybir.dt.np(alloc.dtype)
                out_names.append(name)
                out_avals.append(jax.core.ShapedArray(shape, np_dt))
                zero_outs.append(np.zeros(shape, np_dt))
        self.in_names, self.out_names = in_names, out_names
        self.out_avals, self.zero_outs = out_avals, zero_outs
        all_in = list(in_names) + list(out_names)
        if pname is not None:
            all_in.append(pname)

        def _body(*args):
            operands = list(args)
            if pname is not None:
                operands.append(partition_id_tensor())
            return tuple(_bass_exec_p.bind(
                *operands, out_avals=tuple(out_avals), in_names=tuple(all_in),
                out_names=tuple(out_names), lowering_input_output_aliases=(),
                sim_require_finite=True, sim_require_nnan=True, nc=nc))

        self.n_params = len(in_names)
        devices = jax.devices()[:n_cores]
        mesh = Mesh(np.asarray(devices), ("core",))
        n_outs = len(out_names)
        self.fn = jax.jit(
            shard_map(_body, mesh=mesh,
                      in_specs=(PartitionSpec("core"),) * (self.n_params + n_outs),
                      out_specs=(PartitionSpec("core"),) * n_outs,
                      check_rep=False),
            keep_unused=True)

    def run(self, in_maps):
        import time
        n = self.n_cores
        per_core = [[np.asarray(m[k]) for k in self.in_names] for m in in_maps]
        args = [np.concatenate([per_core[c][i] for c in range(n)], axis=0)
                for i in range(self.n_params)]
        args += [np.zeros((n * z.shape[0], *z.shape[1:]), z.dtype)
                 for z in self.zero_outs]
        t0 = time.time()
        outs = self.fn(*args)
        self.jax.block_until_ready(outs)
        wall = time.time() - t0
        res = [{k: np.asarray(outs[i]).reshape(n, *self.out_avals[i].shape)[c]
                for i, k in enumerate(self.out_names)} for c in range(n)]
        return res, wall


# ----------------------------------------------------------- preprocessing --
def _ceil128(x):
    return (x + 127) // 128 * 128


def _preprocess(node_features, edge_src, edge_dst):
    src = np.asarray(edge_src).astype(np.int64)
    dst = np.asarray(edge_dst).astype(np.int64)
    nf = np.asarray(node_features, dtype=np.float32)

    core = dst // NPC
    dstl = dst % NPC
    win = dstl // WIN
    bkt = src // BK
    key = (core * NB + bkt) * NW + win
    order = np.argsort(key, kind="stable")
    s_src, s_core, s_key = src[order], core[order], key[order]
    s_dwl = (dstl % WIN)[order]          # dst-local within window, 0..127
    s_bw = s_key % (NB * NW)             # (bucket, window) id within core

    counts = np.bincount(key, minlength=NCORES * NB * NW).reshape(NCORES, NB * NW)
    G = _ceil128(counts.max(axis=0))     # shared group sizes [NB*NW]
    # bucket runs padded to a multiple of CH so every gather chunk is full
    EB = G.reshape(NB, NW).sum(axis=1)
    boff = np.concatenate([[0], np.cumsum(EB)])[:-1]
    EBp = (EB + CH - 1) // CH * CH
    boffp = np.concatenate([[0], np.cumsum(EBp)])[:-1]
    goff0 = np.concatenate([[0], np.cumsum(G)])[:-1]
    bkt_of = np.arange(NB * NW) // NW
    goff = goff0 - boff[bkt_of] + boffp[bkt_of]   # group offsets, padded layout
    Ep = int(EBp.sum())

    # rank of each sorted edge within its (core, bucket, window) group
    starts = np.concatenate([[0], np.cumsum(counts.ravel())])[:-1]
    rank = np.arange(len(s_src)) - starts[(s_core * NB * NW + s_bw)]
    pos = goff[s_bw] + rank

    srcl = np.zeros((NCORES, Ep), np.int16)
    dwl = np.full((NCORES, Ep), -1.0, np.float32)
    srcl[s_core, pos] = (s_src % BK).astype(np.int16)
    dwl[s_core, pos] = s_dwl

    # idx wrap: index i -> partition i%16, col i//16; replicate to 128 parts
    iw16 = srcl.reshape(NCORES, Ep // 16, 16).transpose(0, 2, 1)
    iw = np.tile(iw16, (1, 8, 1)).copy()                       # [NC,128,Ep/16]
    # dst-locals in gather layout: partition e%128, col e//128
    dw = dwl.reshape(NCORES, Ep // 128, 128).transpose(0, 2, 1)
    dw = np.ascontiguousarray(dw.astype(BF16))                 # [NC,128,Ep/128]

    # feature table, bf16, padded to 256B rows
    tbl = np.zeros((TBL_ROWS, ES), BF16)
    tbl[:N, :D] = nf.astype(BF16)

    # reciprocal counts per core in [128, NW] layout (partition=dst%128)
    cnt = np.bincount(dst, minlength=N).astype(np.float32)
    rcp = 1.0 / np.maximum(cnt, 1.0)
    rcp_pad = np.zeros((NCORES, NW * WIN), np.float32)
    rcp_pad[:, :NPC] = rcp.reshape(NCORES, NPC)
    rcpw = np.ascontiguousarray(rcp_pad.reshape(NCORES, NW, WIN).transpose(0, 2, 1))

    # static schedule: uniform full-size chunks
    chunks = []                                   # (edge_off, size, bucket)
    for b in range(NB):
        for k in range(int(EBp[b]) // CH):
            chunks.append((int(boffp[b]) + k * CH, CH, b))
    # contiguous accumulation groups: (window, first subtile, last subtile)
    groups = []
    for bw in range(NB * NW):
        g = int(G[bw])
        if g:
            s0 = int(goff[bw]) // 128
            groups.append((bw % NW, s0, s0 + g // 128 - 1))
    return dict(iw=iw, dw=dw, tbl=tbl, rcpw=rcpw, Ep=Ep, chunks=chunks,
                groups=groups)


# ---------------------------------------------------------------- program --
def _build(meta, WT_np, BT_np):
    Ep = meta["Ep"]
    chunks = meta["chunks"]
    groups = meta["groups"]
    gstart = {s0: (w, s1) for (w, s0, s1) in groups}

    nc = bacc.Bacc()
    f_t = nc.dram_tensor("tbl", [TBL_ROWS, ES], mybir.dt.bfloat16, kind="ExternalInput")
    iw_t = nc.dram_tensor("iw", [128, Ep // 16], mybir.dt.int16, kind="ExternalInput")
    dw_t = nc.dram_tensor("dw", [128, Ep // 128], mybir.dt.bfloat16, kind="ExternalInput")
    rcp_t = nc.dram_tensor("rcp", [128, NW], mybir.dt.float32, kind="ExternalInput")
    wt_t = nc.dram_tensor("wt", [D, D], mybir.dt.float32, kind="ExternalInput")
    bt_t = nc.dram_tensor("bt", [D, D], mybir.dt.float32, kind="ExternalInput")
    j_t = nc.dram_tensor("jt", [128, 128], mybir.dt.bfloat16, kind="ExternalInput")
    id_t = nc.dram_tensor("idt", [128, 128], mybir.dt.float32, kind="ExternalInput")
    out_t = nc.dram_tensor("out", [NPC, D], mybir.dt.float32, kind="ExternalOutput")

    with tile.TileContext(nc) as tc:
        with tc.tile_pool(name="const", bufs=1) as cp, \
             tc.tile_pool(name="idxp", bufs=3) as idxp, \
             tc.tile_pool(name="dwp", bufs=3) as dwp, \
             tc.tile_pool(name="gp", bufs=3) as gp, \
             tc.tile_pool(name="ohp", bufs=3) as ohp, \
             tc.tile_pool(name="ep", bufs=4) as ep, \
             tc.tile_pool(name="psA", bufs=4, space="PSUM") as psA, \
             tc.tile_pool(name="psE", bufs=2, space="PSUM") as psE:

            jt = cp.tile([128, 128], mybir.dt.bfloat16, tag="jt")
            nc.sync.dma_start(out=jt[:], in_=j_t[:])
            idt = cp.tile([128, 128], mybir.dt.float32, tag="idt")
            nc.sync.dma_start(out=idt[:], in_=id_t[:])
            rcps = cp.tile([128, NW], mybir.dt.float32, tag="rcp")
            nc.sync.dma_start(out=rcps[:], in_=rcp_t[:])
            wts = cp.tile([D, D], mybir.dt.float32, tag="wt")
            nc.sync.dma_start(out=wts[:], in_=wt_t[:])
            bts = cp.tile([D, D], mybir.dt.float32, tag="bt")
            nc.sync.dma_start(out=bts[:], in_=bt_t[:])
            wct = cp.tile([D, D], mybir.dt.float32, tag="wct")
            nc.vector.tensor_add(out=wct[:], in0=wts[:], in1=bts[:])
            outsb = cp.tile([128, NW * D], mybir.dt.float32, tag="outsb")
            aggsb = cp.tile([128, NW * D], mybir.dt.float32, tag="aggsb")
            nc.vector.memset(aggsb[:], 0.0)
            cur = None  # (window, last_subtile, psum_tile)
            for (eoff, esz, b) in chunks:
                c128 = CH // 128
                it = idxp.tile([128, CH // 16], mybir.dt.int16, tag="it")
                nc.sync.dma_start(out=it[:], in_=iw_t[:, eoff // 16: (eoff + CH) // 16])
                dt_ = dwp.tile([128, CH // 128], mybir.dt.bfloat16, tag="dt")
                nc.sync.dma_start(out=dt_[:], in_=dw_t[:, eoff // 128: (eoff + CH) // 128])
                gt = gp.tile([128, CH // 128, ES], mybir.dt.bfloat16, tag="gt")
                if DEBUG_NO_GATHER:
                    nc.vector.memset(gt[:], 0.5)
                else:
                    nc.gpsimd.dma_gather(
                        gt[:], f_t[b * BK:(b + 1) * BK, :], it[:],
                        CH, CH, ES, elem_step=ES, single_packet=False)
                oh = ohp.tile([128, CH // 128, 128], mybir.dt.bfloat16, tag="oh")
                nc.vector.tensor_tensor(
                    out=oh[:],
                    in0=dt_[:].unsqueeze(2).to_broadcast([128, c128, 128]),
                    in1=jt[:].unsqueeze(1).to_broadcast([128, c128, 128]),
                    op=mybir.AluOpType.is_equal)
                for m in range(c128):
                    s = eoff // 128 + m
                    if s in gstart:
                        w, s1 = gstart[s]
                        pt = psA.tile([128, D], mybir.dt.float32, tag="psg")
                        cur = (w, s1, pt)
                    if cur is None:
                        continue
                    w, s1, pt = cur
                    nc.tensor.matmul(
                        out=pt[:], lhsT=oh[:, m, :], rhs=gt[:, m, :D],
                        start=(s in gstart), stop=(s == s1),
                        skip_group_check=True)
                    if s == s1:
                        osl_ = aggsb[:, w * D:(w + 1) * D]
                        nc.vector.tensor_add(out=osl_, in0=osl_, in1=pt[:])
                        cur = None

            # epilogue per window: mean, linear, relu
            for w in range(NW):
                osl = outsb[:, w * D:(w + 1) * D]
                if DEBUG_NO_LINEAR:
                    nc.vector.tensor_copy(out=osl, in_=aggsb[:, w * D:(w + 1) * D])
                    continue
                tps = psE.tile([D, 128], mybir.dt.float32, tag="tps")
                nc.tensor.transpose(out=tps[:], in_=aggsb[:, w * D:(w + 1) * D],
                                    identity=idt[:])
                atc = ep.tile([D, 128], mybir.dt.float32, tag="atc")
                nc.vector.tensor_copy(out=atc[:], in_=tps[:])
                op_ = psE.tile([128, D], mybir.dt.float32, tag="op")
                nc.tensor.matmul(out=op_[:], lhsT=atc[:], rhs=wct[:],
                                 start=True, stop=True, skip_group_check=True)
                nc.vector.tensor_tensor(
                    out=osl, in0=op_[:],
                    in1=rcps[:, w:w + 1].to_broadcast([128, D]),
                    op=mybir.AluOpType.mult)
                nc.vector.tensor_scalar(out=osl, in0=osl, scalar1=0.0,
                                        scalar2=None, op0=mybir.AluOpType.max)

            nfull = NPC // 128            # 195 full windows
            rem = NPC - nfull * 128       # 40
            if nfull:
                nc.sync.dma_start(
                    out=out_t[:nfull * 128, :].rearrange("(w p) f -> p w f", p=128),
                    in_=outsb[:, :nfull * D])
            if rem:
                nc.sync.dma_start(
                    out=out_t[nfull * 128:, :],
                    in_=outsb[:rem, nfull * D:(nfull + 1) * D])
    nc.compile()
    return nc


_CACHE = {}


def kernel(node_features, W, B, edge_src, edge_dst):
    _install_bir_fix()
    meta = _preprocess(node_features, edge_src, edge_dst)
    WT = np.asarray(W, np.float32).T.copy()
    BT = np.asarray(B, np.float32).T.copy()
    sig = (meta["Ep"], tuple(meta["chunks"]))
    if sig not in _CACHE:
        nc = _build(meta, WT, BT)
        _CACHE[sig] = _Runner(nc, NCORES)
    runner = _CACHE[sig]

    jmat = np.tile(np.arange(128, dtype=np.float32).astype(BF16), (128, 1))
    ident = np.eye(128, dtype=np.float32)
    in_maps = []
    for c in range(NCORES):
        in_maps.append({
            "tbl": meta["tbl"], "iw": meta["iw"][c], "dw": meta["dw"][c],
            "rcp": meta["rcpw"][c], "wt": WT, "bt": BT,
            "jt": jmat, "idt": ident,
        })
    res, wall = runner.run(in_maps)
    kernel.last_exec_wall = wall
    out = np.concatenate([res[c]["out"] for c in range(NCORES)], axis=0)
    return out.astype(np.float32)


# revision 10
# speedup vs baseline: 1.2210x; 1.2210x over previous
"""Trainium2 kernel for gnn_message_passing (nn_MessagePassing_41480794145043).

out = relu((segment_mean of node_features[edge_src] over edge_dst) @ (W+B)^T)

Strategy (8 NeuronCores, SPMD single program):
  - Destination nodes sharded across cores (25000 dst rows each).
  - Host: sorts each core's edges by (src_bucket, dst_window128), pads each
    (bucket, window) group to a multiple of 128 with identical group sizes on
    every core so one static program serves all cores.
  - Device: per 8192-edge chunk, dma_gather (GatherAnt, bf16 table padded to
    256B rows) pulls source features; a single broadcast is_equal builds all
    the [128e x 128d] one-hot tiles; one matmul per 128-edge subtile
    accumulates agg^T windows in PSUM (all 196 windows resident).
  - Epilogue per window: PSUM->SBUF, PE transpose, small matmul with (W+B)^T,
    scale by 1/max(count,1), relu, one big DMA out.
"""
import sys
import json
import numpy as np

sys.path.insert(0, '/opt/trn_rl_repo')

import ml_dtypes  # noqa: E402
import concourse.bass as bass  # noqa: E402
import concourse.mybir as mybir  # noqa: E402
import concourse.tile as tile  # noqa: E402
from concourse import bacc  # noqa: E402

BF16 = ml_dtypes.bfloat16

N = 200000
E = 12800000
D = 10
NCORES = 8
NPC = N // NCORES            # dst nodes per core
WIN = 128                    # dst window (matmul M)
NW = (NPC + WIN - 1) // WIN  # windows per core (196 for 25000)
BK = 32768                   # src bucket size (int16 index range)
NB = (N + BK - 1) // BK      # 7 buckets
CH = 8192                    # edges per gather chunk
TBL_ROWS = NB * BK           # padded feature table rows
ES = 128                     # gathered elem size in bf16 (= 256B)
WPB = 51                     # windows per PSUM bank (51*10=510 <= 512 fp32)
DEBUG_NO_LINEAR = False
DEBUG_NO_GATHER = False


# ----------------------------------------------------------------- BIR fix --
def _fix_bir_json(bir_json: bytes) -> bytes:
    """This walrus build accepts max one sem-wait per instruction; split
    multi-wait instructions into preceding single-wait EventSemaphore nops."""
    d = json.loads(bir_json)
    for fn in d.get("functions", []):
        for bb in fn.get("blocks", []):
            out = []
            for ins in bb.get("instructions", []):
                sync = ins.get("sync_info")
                waits = (sync or {}).get("on_wait") or []
                if len(waits) > 1:
                    for i, w in enumerate(waits[:-1]):
                        out.append({
                            "debug": ins.get("debug", 0),
                            "engine": ins["engine"],
                            "ins": [], "outs": [],
                            "name": f"{ins['name']}-wsplit{i}",
                            "opcode": "EventSemaphore",
                            "sync_info": {"on_update": [], "on_wait": [w]},
                        })
                    sync["on_wait"] = [waits[-1]]
                out.append(ins)
            bb["instructions"] = out
    return json.dumps(d).encode()


def _install_bir_fix():
    import concourse.bass_utils as bu
    import concourse.bass2jax as b2j
    orig = bu.compile_bir_kernel
    if getattr(orig, "_bir_fix_installed", False):
        return
    def wrapped(bir_json, tmpdir, neff_name="file.neff"):
        return orig(_fix_bir_json(bir_json), tmpdir, neff_name=neff_name)
    wrapped._bir_fix_installed = True
    bu.compile_bir_kernel = wrapped
    b2j.compile_bir_kernel = wrapped


# ------------------------------------------------------------------ runner --
class _Runner:
    def __init__(self, nc, n_cores, replicated=()):
        import jax
        from jax.sharding import Mesh, PartitionSpec
        from jax.experimental.shard_map import shard_map
        from concourse.bass2jax import (_bass_exec_p, install_neuronx_cc_hook,
                                        partition_id_tensor)
        install_neuronx_cc_hook()
        self.jax = jax
        self.n_cores = n_cores
        in_names, out_names, out_avals, zero_outs = [], [], [], []
        pname = nc.partition_id_tensor.name if nc.partition_id_tensor else None
        for alloc in nc.m.functions[0].allocations:
            if not isinstance(alloc, mybir.MemoryLocationSet):
                continue
            name = alloc.memorylocations[0].name
            if alloc.kind == "ExternalInput":
                if name != pname:
                    in_names.append(name)
            elif alloc.kind == "ExternalOutput":
                shape = list(alloc.tensor_shape)
                np_dt = mybir.dt.np(alloc.dtype)
                out_names.append(name)
                out_avals.append(jax.core.ShapedArray(shape, np_dt))
                zero_outs.append(np.zeros(shape, np_dt))
        self.in_names, self.out_names = in_names, out_names
        self.out_avals, self.zero_outs = out_avals, zero_outs
        all_in = list(in_names) + list(out_names)
        if pname is not None:
            all_in.append(pname)

        def _body(*args):
            operands = list(args)
            if pname is not None:
                operands.append(partition_id_tensor())
            return tuple(_bass_exec_p.bind(
                *operands, out_avals=tuple(out_avals), in_names=tuple(all_in),
                out_names=tuple(out_names), lowering_input_output_aliases=(),
                sim_require_finite=True, sim_require_nnan=True, nc=nc))

        self.n_params = len(in_names)
        self.replicated = set(replicated)
        devices = jax.devices()[:n_cores]
        mesh = Mesh(np.asarray(devices), ("core",))
        n_outs = len(out_names)
        in_specs = tuple(
            PartitionSpec() if nm in self.replicated else PartitionSpec("core")
            for nm in in_names) + (PartitionSpec("core"),) * n_outs
        self.fn = jax.jit(
            shard_map(_body, mesh=mesh, in_specs=in_specs,
                      out_specs=(PartitionSpec("core"),) * n_outs,
                      check_rep=False),
            keep_unused=True)

    def run(self, in_maps):
        import time
        n = self.n_cores
        per_core = [[np.asarray(m[k]) for k in self.in_names] for m in in_maps]
        args = [per_core[0][i] if self.in_names[i] in self.replicated
                else np.concatenate([per_core[c][i] for c in range(n)], axis=0)
                for i in range(self.n_params)]
        args += [np.zeros((n * z.shape[0], *z.shape[1:]), z.dtype)
                 for z in self.zero_outs]
        t0 = time.time()
        outs = self.fn(*args)
        self.jax.block_until_ready(outs)
        wall = time.time() - t0
        res = [{k: np.asarray(outs[i]).reshape(n, *self.out_avals[i].shape)[c]
                for i, k in enumerate(self.out_names)} for c in range(n)]
        return res, wall


# ----------------------------------------------------------- preprocessing --
def _ceil128(x):
    return (x + 127) // 128 * 128


def _preprocess(node_features, edge_src, edge_dst):
    src = np.asarray(edge_src).astype(np.int64)
    dst = np.asarray(edge_dst).astype(np.int64)
    nf = np.asarray(node_features, dtype=np.float32)

    core = dst // NPC
    dstl = dst % NPC
    win = dstl // WIN
    bkt = src // BK
    key = (core * NB + bkt) * NW + win
    order = np.argsort(key, kind="stable")
    s_src, s_core, s_key = src[order], core[order], key[order]
    s_dwl = (dstl % WIN)[order]          # dst-local within window, 0..127
    s_bw = s_key % (NB * NW)             # (bucket, window) id within core

    counts = np.bincount(key, minlength=NCORES * NB * NW).reshape(NCORES, NB * NW)
    G = _ceil128(counts.max(axis=0))     # shared group sizes [NB*NW]
    # bucket runs padded to a multiple of CH so every gather chunk is full
    EB = G.reshape(NB, NW).sum(axis=1)
    boff = np.concatenate([[0], np.cumsum(EB)])[:-1]
    EBp = (EB + CH - 1) // CH * CH
    boffp = np.concatenate([[0], np.cumsum(EBp)])[:-1]
    goff0 = np.concatenate([[0], np.cumsum(G)])[:-1]
    bkt_of = np.arange(NB * NW) // NW
    goff = goff0 - boff[bkt_of] + boffp[bkt_of]   # group offsets, padded layout
    Ep = int(EBp.sum())

    # rank of each sorted edge within its (core, bucket, window) group
    starts = np.concatenate([[0], np.cumsum(counts.ravel())])[:-1]
    rank = np.arange(len(s_src)) - starts[(s_core * NB * NW + s_bw)]
    pos = goff[s_bw] + rank

    srcl = np.zeros((NCORES, Ep), np.int16)
    dwl = np.full((NCORES, Ep), -1.0, np.float32)
    srcl[s_core, pos] = (s_src % BK).astype(np.int16)
    dwl[s_core, pos] = s_dwl

    # idx wrap: index i -> partition i%16, col i//16; replicate to 128 parts
    iw16 = srcl.reshape(NCORES, Ep // 16, 16).transpose(0, 2, 1)
    iw = np.tile(iw16, (1, 8, 1)).copy()                       # [NC,128,Ep/16]
    # dst-locals in gather layout: partition e%128, col e//128
    dw = dwl.reshape(NCORES, Ep // 128, 128).transpose(0, 2, 1)
    dw = np.ascontiguousarray(dw.astype(BF16))                 # [NC,128,Ep/128]

    # feature table, bf16, padded to 256B rows
    tbl = np.zeros((TBL_ROWS, ES), BF16)
    tbl[:N, :D] = nf.astype(BF16)

    # reciprocal counts per core in [128, NW] layout (partition=dst%128)
    cnt = np.bincount(dst, minlength=N).astype(np.float32)
    rcp = 1.0 / np.maximum(cnt, 1.0)
    rcp_pad = np.zeros((NCORES, NW * WIN), np.float32)
    rcp_pad[:, :NPC] = rcp.reshape(NCORES, NPC)
    rcpw = np.ascontiguousarray(rcp_pad.reshape(NCORES, NW, WIN).transpose(0, 2, 1))

    # static schedule: uniform full-size chunks
    chunks = []                                   # (edge_off, size, bucket)
    for b in range(NB):
        for k in range(int(EBp[b]) // CH):
            chunks.append((int(boffp[b]) + k * CH, CH, b))
    # contiguous accumulation groups: (window, first subtile, last subtile)
    groups = []
    for bw in range(NB * NW):
        g = int(G[bw])
        if g:
            s0 = int(goff[bw]) // 128
            groups.append((bw % NW, s0, s0 + g // 128 - 1))
    return dict(iw=iw, dw=dw, tbl=tbl, rcpw=rcpw, Ep=Ep, chunks=chunks,
                groups=groups)


# ---------------------------------------------------------------- program --
def _build(meta, WT_np, BT_np):
    Ep = meta["Ep"]
    chunks = meta["chunks"]
    groups = meta["groups"]
    gstart = {s0: (w, s1) for (w, s0, s1) in groups}

    nc = bacc.Bacc()
    f_t = nc.dram_tensor("tbl", [TBL_ROWS, ES], mybir.dt.bfloat16, kind="ExternalInput")
    iw_t = nc.dram_tensor("iw", [128, Ep // 16], mybir.dt.int16, kind="ExternalInput")
    dw_t = nc.dram_tensor("dw", [128, Ep // 128], mybir.dt.bfloat16, kind="ExternalInput")
    rcp_t = nc.dram_tensor("rcp", [128, NW], mybir.dt.float32, kind="ExternalInput")
    wt_t = nc.dram_tensor("wt", [D, D], mybir.dt.float32, kind="ExternalInput")
    bt_t = nc.dram_tensor("bt", [D, D], mybir.dt.float32, kind="ExternalInput")
    j_t = nc.dram_tensor("jt", [128, 128], mybir.dt.bfloat16, kind="ExternalInput")
    id_t = nc.dram_tensor("idt", [128, 128], mybir.dt.float32, kind="ExternalInput")
    out_t = nc.dram_tensor("out", [NPC, D], mybir.dt.float32, kind="ExternalOutput")

    with tile.TileContext(nc) as tc:
        with tc.tile_pool(name="const", bufs=1) as cp, \
             tc.tile_pool(name="idxp", bufs=3) as idxp, \
             tc.tile_pool(name="dwp", bufs=3) as dwp, \
             tc.tile_pool(name="gp", bufs=3) as gp, \
             tc.tile_pool(name="ohp", bufs=3) as ohp, \
             tc.tile_pool(name="ep", bufs=4) as ep, \
             tc.tile_pool(name="psA", bufs=4, space="PSUM") as psA, \
             tc.tile_pool(name="psE", bufs=2, space="PSUM") as psE:

            jt = cp.tile([128, 128], mybir.dt.bfloat16, tag="jt")
            nc.sync.dma_start(out=jt[:], in_=j_t[:])
            idt = cp.tile([128, 128], mybir.dt.float32, tag="idt")
            nc.sync.dma_start(out=idt[:], in_=id_t[:])
            rcps = cp.tile([128, NW], mybir.dt.float32, tag="rcp")
            nc.sync.dma_start(out=rcps[:], in_=rcp_t[:])
            wts = cp.tile([D, D], mybir.dt.float32, tag="wt")
            nc.sync.dma_start(out=wts[:], in_=wt_t[:])
            bts = cp.tile([D, D], mybir.dt.float32, tag="bt")
            nc.sync.dma_start(out=bts[:], in_=bt_t[:])
            wct = cp.tile([D, D], mybir.dt.float32, tag="wct")
            nc.vector.tensor_add(out=wct[:], in0=wts[:], in1=bts[:])
            outsb = cp.tile([128, NW * D], mybir.dt.float32, tag="outsb")
            aggsb = cp.tile([128, NW * D], mybir.dt.float32, tag="aggsb")
            nc.vector.memset(aggsb[:], 0.0)
            cur = None  # (window, last_subtile, psum_tile)
            for (eoff, esz, b) in chunks:
                c128 = CH // 128
                it = idxp.tile([128, CH // 16], mybir.dt.int16, tag="it")
                nc.sync.dma_start(out=it[:], in_=iw_t[:, eoff // 16: (eoff + CH) // 16])
                dt_ = dwp.tile([128, CH // 128], mybir.dt.bfloat16, tag="dt")
                nc.sync.dma_start(out=dt_[:], in_=dw_t[:, eoff // 128: (eoff + CH) // 128])
                gt = gp.tile([128, CH // 128, ES], mybir.dt.bfloat16, tag="gt")
                if DEBUG_NO_GATHER:
                    nc.vector.memset(gt[:], 0.5)
                else:
                    nc.gpsimd.dma_gather(
                        gt[:], f_t[b * BK:(b + 1) * BK, :], it[:],
                        CH, CH, ES, elem_step=ES, single_packet=False)
                oh = ohp.tile([128, CH // 128, 128], mybir.dt.bfloat16, tag="oh")
                nc.vector.tensor_tensor(
                    out=oh[:],
                    in0=dt_[:].unsqueeze(2).to_broadcast([128, c128, 128]),
                    in1=jt[:].unsqueeze(1).to_broadcast([128, c128, 128]),
                    op=mybir.AluOpType.is_equal)
                for m in range(c128):
                    s = eoff // 128 + m
                    if s in gstart:
                        w, s1 = gstart[s]
                        pt = psA.tile([128, D], mybir.dt.float32, tag="psg")
                        cur = (w, s1, pt)
                    if cur is None:
                        continue
                    w, s1, pt = cur
                    nc.tensor.matmul(
                        out=pt[:], lhsT=oh[:, m, :], rhs=gt[:, m, :D],
                        start=(s in gstart), stop=(s == s1),
                        skip_group_check=True)
                    if s == s1:
                        osl_ = aggsb[:, w * D:(w + 1) * D]
                        nc.vector.tensor_add(out=osl_, in0=osl_, in1=pt[:])
                        cur = None

            # epilogue per window: mean, linear, relu
            for w in range(NW):
                osl = outsb[:, w * D:(w + 1) * D]
                if DEBUG_NO_LINEAR:
                    nc.vector.tensor_copy(out=osl, in_=aggsb[:, w * D:(w + 1) * D])
                    continue
                tps = psE.tile([D, 128], mybir.dt.float32, tag="tps")
                nc.tensor.transpose(out=tps[:], in_=aggsb[:, w * D:(w + 1) * D],
                                    identity=idt[:])
                atc = ep.tile([D, 128], mybir.dt.float32, tag="atc")
                nc.vector.tensor_copy(out=atc[:], in_=tps[:])
                op_ = psE.tile([128, D], mybir.dt.float32, tag="op")
                nc.tensor.matmul(out=op_[:], lhsT=atc[:], rhs=wct[:],
                                 start=True, stop=True, skip_group_check=True)
                nc.vector.tensor_tensor(
                    out=osl, in0=op_[:],
                    in1=rcps[:, w:w + 1].to_broadcast([128, D]),
                    op=mybir.AluOpType.mult)
                nc.vector.tensor_scalar(out=osl, in0=osl, scalar1=0.0,
                                        scalar2=None, op0=mybir.AluOpType.max)

            nfull = NPC // 128            # 195 full windows
            rem = NPC - nfull * 128       # 40
            if nfull:
                nc.sync.dma_start(
                    out=out_t[:nfull * 128, :].rearrange("(w p) f -> p w f", p=128),
                    in_=outsb[:, :nfull * D])
            if rem:
                nc.sync.dma_start(
                    out=out_t[nfull * 128:, :],
                    in_=outsb[:rem, nfull * D:(nfull + 1) * D])
    nc.compile()
    return nc


_CACHE = {}


def kernel(node_features, W, B, edge_src, edge_dst):
    _install_bir_fix()
    meta = _preprocess(node_features, edge_src, edge_dst)
    WT = np.asarray(W, np.float32).T.copy()
    BT = np.asarray(B, np.float32).T.copy()
    sig = (meta["Ep"], tuple(meta["chunks"]))
    if sig not in _CACHE:
        nc = _build(meta, WT, BT)
        _CACHE[sig] = _Runner(nc, NCORES,
            replicated=("tbl", "wt", "bt", "jt", "idt"))
    runner = _CACHE[sig]

    jmat = np.tile(np.arange(128, dtype=np.float32).astype(BF16), (128, 1))
    ident = np.eye(128, dtype=np.float32)
    in_maps = []
    for c in range(NCORES):
        in_maps.append({
            "tbl": meta["tbl"], "iw": meta["iw"][c], "dw": meta["dw"][c],
            "rcp": meta["rcpw"][c], "wt": WT, "bt": BT,
            "jt": jmat, "idt": ident,
        })
    res, wall = runner.run(in_maps)
    kernel.last_exec_wall = wall
    out = np.concatenate([res[c]["out"] for c in range(NCORES)], axis=0)
    return out.astype(np.float32)
